# revision 9
# baseline (speedup 1.0000x reference)
"""Trainium2 Bass kernel for 3x3 VALID conv: x[32,128,64,64] * w[256,128,3,3] + bias.

Strategy (v4 — 1D Winograd F(2,3) along H):
  - Data-parallel over batch: 8 cores x 4 images; weights/bias replicated.
  - Winograd F(2,3) on the H axis cuts PE matmul columns by 1/3 vs direct:
    for each row-tile it (2 output rows), 4 transformed inputs
        t0 = x[2it]   - x[2it+2]
        t1 = x[2it+1] + x[2it+2]
        t2 = x[2it+2] - x[2it+1]
        t3 = x[2it+1] - x[2it+3]
    feed 4 point-GEMMs (contraction C_IN=128 on partitions, W-taps v=0..2
    accumulated in PSUM):  M_p[o, it, j] = sum_v sum_c gw[p,v][c,o] t_p[c,it,j+v]
    with gw = G @ w along u (host-side, fp32 then bf16).
    Outputs:  y[2it]   = m0 + (m1+bias) + m2
              y[2it+1] = (m1+bias) - m2 - m3
  - Engine split per (chunk of <=8 row-tiles, half of C_OUT):
      PE:  12 matmuls of N=rn*62 cols (4 points x 3 taps)
      ACT: two activations evacuate PSUM->SBUF bf16 — {M1 + per-partition
           bias} (Identity) and {M0,M2,M3} (Copy); PSUM point order
           m1,m0,m2,m3 so the 3-point evac is one strided AP
      DVE: input transform (4 tensor_tensor) + 4 plain tensor_tensor
           combines (scalar_tensor_tensor has no 2x uop — avoid it)
  - PSUM: 2 groups x [128,4,512] fp32 = 8 banks exactly; groups alternate
    so evacuation overlaps the next group's matmuls. Each sub-split range
    gets its OWN group (sharing banks would serialize MMs behind evac).
  - Startup: the host pre-transforms img0 chunk0 (t00 rides Sync's first
    batch) so the first matmuls need no DVE work; half0 taps ride the
    Scalar ring in parallel; dummy matmuls ramp the PE clock until then
    and a dummy activation preloads the ACT table set.
  - Tail: the last chunk is sub-split (h0: 4+3, h1: 3+2+2 tile-rows) so the
    final evac+combine+store chain is short; the last store rides Sync
    (HWDGE, lower first-byte latency than GpSimd's SWDGE).
  - DMA rings: Sync = t00 + w1 + half0 stores; Scalar = w0 + piece(0,1)
    then pure evacuation compute; GpSimd = bias + remaining pieces +
    half1 stores.
"""

import numpy as np
import ml_dtypes

import concourse.bacc as bacc
import concourse.tile as tile
from concourse import mybir
from concourse.bass_utils import run_bass_kernel_spmd

N_CORES = 8
B_FULL, C_IN, H, W = 32, 128, 64, 64
C_OUT, KH, KW = 256, 3, 3
B_LOC = B_FULL // N_CORES          # images per core
H_OUT = W_OUT = H - KH + 1         # 62
N_HALF = C_OUT // 128              # 2 output-channel halves
NT = H_OUT // 2                    # 31 row-tiles (2 output rows each)
CHUNKS = [(0, 8), (8, 8), (16, 8), (24, 7)]   # (first tile, tiles in chunk)
N_PTS = 4                          # F(2,3) points
SLOT = {1: 0, 0: 1, 2: 2, 3: 3}    # PSUM/evac point order: m1, m0, m2, m3
N_WARM = 6                         # DVFS warm-up matmuls
GW_LEN = N_PTS * KW * 128          # 1536 per half
T0_LEN = N_PTS * 8 * W             # pre-transformed img0 chunk0

_cached = {}


def _build_nc():
    f32 = mybir.dt.float32
    bf16 = mybir.dt.bfloat16
    AF = mybir.ActivationFunctionType
    ALU = mybir.AluOpType
    nc = bacc.Bacc()

    t00_d = nc.declare_dram_parameter("t00", [C_IN, T0_LEN], bf16, isOutput=False)
    w0_d = nc.declare_dram_parameter("w0", [C_IN, GW_LEN], bf16, isOutput=False)
    w1_d = nc.declare_dram_parameter("w1", [C_IN, GW_LEN], bf16, isOutput=False)
    x_d = nc.declare_dram_parameter("x", [B_LOC, C_IN, H, W], bf16, isOutput=False)
    b_d = nc.declare_dram_parameter("bias_in", [128, N_HALF], f32, isOutput=False)
    y_d = nc.declare_dram_parameter(
        "y", [B_LOC, N_HALF, 128, H_OUT, W_OUT], bf16, isOutput=True
    )

    with tile.TileContext(nc) as tc:
        with (
            tc.tile_pool(name="const", bufs=1) as cpool,
            tc.tile_pool(name="xin", bufs=5) as xpool,
            tc.tile_pool(name="tin", bufs=8) as tpool,
            tc.tile_pool(name="mev", bufs=4) as mpool,
            tc.tile_pool(name="yout", bufs=5) as ypool,
            tc.tile_pool(name="tmp", bufs=4) as spool,
            tc.tile_pool(name="psum", bufs=2, space="PSUM") as ppool,
        ):
            t00_t = cpool.tile([C_IN, N_PTS, 8, W], bf16)
            w0_t = cpool.tile([C_IN, GW_LEN], bf16)
            w1_t = cpool.tile([C_IN, GW_LEN], bf16)
            b_t = cpool.tile([128, N_HALF], f32)
            scr = cpool.tile([128, 512], bf16)
            actw = cpool.tile([128, 8], bf16)

            nc.vector.memset(scr[:], 0.0)

            # Startup DMAs: pre-transformed chunk0 and half0 taps land in
            # parallel on the two HWDGE rings; bias leads GpSimd.
            nc.sync.dma_start(
                t00_t[:], t00_d.rearrange("c (p h w) -> c p h w", p=N_PTS, w=W)
            )
            nc.scalar.dma_start(w0_t[:], w0_d[:])
            nc.sync.dma_start(w1_t[:], w1_d[:])
            nc.gpsimd.dma_start(b_t[:], b_d[:])

            # Preload the ACT function-table set off the critical path.
            nc.scalar.activation(actw[:], scr[:, 0:8], AF.Copy)

            gw0 = w0_t.rearrange("c (t o) -> c t o", t=N_PTS * KW)
            gw1 = w1_t.rearrange("c (t o) -> c t o", t=N_PTS * KW)

            def lhsT(half, p, v):
                gv = gw0 if half == 0 else gw1
                return gv[:, p * KW + v, :]

            # Ramp the PE p-state while the startup DMAs land. Warm-ups
            # write the first PSUM group buffer (its first real user is the
            # 3rd group, long after these complete). Distinct sizes so no
            # two warm-up matmuls are identical instructions.
            pwarm = ppool.tile([128, N_PTS, 512], f32, tag="ps")
            for i in range(N_WARM):
                nc.tensor.matmul(
                    pwarm[:, i % N_PTS, 0 : 512 - i],
                    scr[:, 0:128],
                    scr[:, 0 : 512 - i],
                    start=True,
                    stop=True,
                )

            piece = {}

            def load_piece(b, c, eng):
                r0 = 16 * c
                r1 = min(r0 + 18, H)
                px = xpool.tile([C_IN, 18, W], bf16, tag="x")
                eng.dma_start(px[:, 0 : r1 - r0, :], x_d[b, :, r0:r1, :])
                piece[(b, c)] = px

            load_piece(0, 1, nc.scalar)
            load_piece(0, 2, nc.gpsimd)
            load_piece(0, 3, nc.gpsimd)

            def transform(b, c):
                # DVE input transform for one chunk: 4 point-tiles.
                px = piece[(b, c)]
                ctn = CHUNKS[c][1]
                ts = [
                    tpool.tile([C_IN, 8, W], bf16, tag="t", name=f"t{p}")
                    for p in range(N_PTS)
                ]
                r = lambda a: px[:, a : a + 2 * ctn - 1 : 2, :]
                nc.vector.tensor_tensor(ts[0][:, 0:ctn, :], r(0), r(2), ALU.subtract)
                nc.vector.tensor_tensor(ts[1][:, 0:ctn, :], r(1), r(2), ALU.add)
                nc.vector.tensor_tensor(ts[2][:, 0:ctn, :], r(2), r(1), ALU.subtract)
                nc.vector.tensor_tensor(ts[3][:, 0:ctn, :], r(1), r(3), ALU.subtract)
                return ts

            def do_group(b, half, it0, rn, ts, tr0, store_eng):
                # One PSUM group: rn row-tiles starting at absolute tile it0,
                # reading ts[p] rows [tr0, tr0+rn).
                n = rn * W_OUT
                ps = ppool.tile([128, N_PTS, 512], f32, tag="ps")
                m = mpool.tile([128, N_PTS, 8 * W_OUT], bf16, tag="m")
                yt = ypool.tile([128, 16, W_OUT], bf16, tag="y")
                bh = b_t[:, half : half + 1]
                for p in range(N_PTS):
                    for v in range(KW):
                        nc.tensor.matmul(
                            ps[:, SLOT[p], 0:n],
                            lhsT(half, p, v),
                            ts[p][:, tr0 : tr0 + rn, v : v + W_OUT],
                            start=(v == 0),
                            stop=(v == KW - 1),
                        )
                # Evacuate PSUM->SBUF bf16: {m1 + bias}, then {m0,m2,m3}.
                nc.scalar.activation(m[:, 0, 0:n], ps[:, 0, 0:n], AF.Identity, bias=bh)
                nc.scalar.activation(m[:, 1:4, 0:n], ps[:, 1:4, 0:n], AF.Copy)
                m3d = [
                    m[:, SLOT[p], 0:n].rearrange("c (h w) -> c h w", w=W_OUT)
                    for p in range(N_PTS)
                ]
                y0 = yt[:, 0 : 2 * rn : 2, :]
                y1 = yt[:, 1 : 2 * rn : 2, :]
                s = spool.tile([128, 8, W_OUT], bf16, tag="s")
                sv = s[:, 0:rn, :]
                nc.vector.tensor_tensor(sv, m3d[0], m3d[1], ALU.add)
                nc.vector.tensor_tensor(y0, sv, m3d[2], ALU.add)
                u = spool.tile([128, 8, W_OUT], bf16, tag="s")
                uv = u[:, 0:rn, :]
                nc.vector.tensor_tensor(uv, m3d[1], m3d[2], ALU.subtract)
                nc.vector.tensor_tensor(y1, uv, m3d[3], ALU.subtract)
                store_eng.dma_start(
                    y_d[b, half, :, 2 * it0 : 2 * (it0 + rn), :],
                    yt[:, 0 : 2 * rn, :],
                )

            t00v = [t00_t[:, p, :, :] for p in range(N_PTS)]
            order = [(b, c) for b in range(B_LOC) for c in range(len(CHUNKS))]
            ts_cur = t00v
            for gi, (b, c) in enumerate(order):
                if gi + 2 < len(order) and order[gi + 2] not in piece:
                    load_piece(*order[gi + 2], nc.gpsimd)
                ts_next = transform(*order[gi + 1]) if gi + 1 < len(order) else None
                it0, ctn = CHUNKS[c]
                if gi < len(order) - 1:
                    do_group(b, 0, it0, ctn, ts_cur, 0, nc.sync)
                    do_group(b, 1, it0, ctn, ts_cur, 0, nc.gpsimd)
                else:
                    # Tail: short final groups, last store on Sync (HWDGE).
                    for r0, rn in [(0, 4), (4, 3)]:
                        do_group(b, 0, it0 + r0, rn, ts_cur, r0, nc.sync)
                    for r0, rn in [(0, 3), (3, 2)]:
                        do_group(b, 1, it0 + r0, rn, ts_cur, r0, nc.gpsimd)
                    do_group(b, 1, it0 + 5, 2, ts_cur, 5, nc.sync)
                ts_cur = ts_next

    nc.compile()
    if not nc.is_finalized():
        nc.finalize()
    return nc


_G = np.array(
    [[1.0, 0.0, 0.0], [0.5, 0.5, 0.5], [0.5, -0.5, 0.5], [0.0, 0.0, 1.0]],
    dtype=np.float32,
)


def kernel(inputs, weights, bias, profile=False, trace_kwargs=None):
    x_b = np.ascontiguousarray(
        np.asarray(inputs, dtype=np.float32).astype(ml_dtypes.bfloat16)
    )
    # gw[p,v][c,o]: Winograd-transformed taps, [c, half, p*3+v, o_local]
    w = np.asarray(weights, dtype=np.float32)
    gw = np.einsum("pu,ocuv->cpvo", _G, w)          # [128, 4, 3, 256]
    gwh = (
        gw.reshape(C_IN, N_PTS * KW, N_HALF, 128)
        .transpose(0, 2, 1, 3)
        .astype(ml_dtypes.bfloat16)
    )                                                # [128, 2, 12, 128]
    w0_flat = np.ascontiguousarray(gwh[:, 0].reshape(C_IN, GW_LEN))
    w1_flat = np.ascontiguousarray(gwh[:, 1].reshape(C_IN, GW_LEN))
    # [C_OUT, 1] -> [128, N_HALF] with b_t[p, h] = bias[h*128 + p]
    b_t = np.ascontiguousarray(
        np.asarray(bias, dtype=np.float32).reshape(N_HALF, 128).T
    )

    if "nc" not in _cached:
        _cached["nc"] = _build_nc()
    nc = _cached["nc"]

    in_maps = []
    for i in range(N_CORES):
        shard = x_b[i * B_LOC : (i + 1) * B_LOC]
        # Host-side input transform of img0 chunk0 (tiles 0..7), matching
        # the on-device math: bf16 inputs, fp32 adds, bf16 result.
        # t0 = x[2k] - x[2k+2]; t1 = x[2k+1] + x[2k+2];
        # t2 = x[2k+2] - x[2k+1]; t3 = x[2k+1] - x[2k+3]   (k = 0..7)
        xr = shard[0, :, 0:18, :].astype(np.float32)     # [128, 18, 64]
        x2k = xr[:, 0:15:2]                              # rows 0,2,..,14
        x2k1 = xr[:, 1:16:2]                             # rows 1,3,..,15
        x2k2 = xr[:, 2:17:2]                             # rows 2,4,..,16
        x2k3 = xr[:, 3:18:2]                             # rows 3,5,..,17
        t00 = np.stack(
            [x2k - x2k2, x2k1 + x2k2, x2k2 - x2k1, x2k1 - x2k3], axis=1
        ).astype(ml_dtypes.bfloat16)                     # [128, 4, 8, 64]
        t00_flat = np.ascontiguousarray(t00.reshape(C_IN, T0_LEN))
        in_maps.append(
            {
                "t00": t00_flat,
                "w0": w0_flat,
                "w1": w1_flat,
                "x": shard,
                "bias_in": b_t,
            }
        )
    res = run_bass_kernel_spmd(
        nc,
        in_maps,
        list(range(N_CORES)),
        trace=profile,
        **(trace_kwargs or {}),
    )
    _cached["last_result"] = res

    shards = []
    for i in range(N_CORES):
        y = res.results[i]["y"]  # [B_LOC, 2, 128, 62, 62] bf16
        shards.append(
            np.asarray(y).astype(np.float32).reshape(B_LOC, C_OUT, H_OUT, W_OUT)
        )
    return np.ascontiguousarray(np.concatenate(shards, axis=0), dtype=np.float32)


# revision 10
# speedup vs baseline: 1.0049x; 1.0049x over previous
"""Trainium2 Bass kernel for 3x3 VALID conv: x[32,128,64,64] * w[256,128,3,3] + bias.

Strategy (v5 — 1D Winograd F(2,3) along H):
  - Data-parallel over batch: 8 cores x 4 images; weights/bias replicated.
  - Winograd F(2,3) on the H axis cuts PE matmul columns by 1/3 vs direct:
    for each row-tile it (2 output rows), 4 transformed inputs
        t0 = x[2it]   - x[2it+2]
        t1 = x[2it+1] + x[2it+2]
        t2 = x[2it+2] - x[2it+1]
        t3 = x[2it+1] - x[2it+3]
    feed 4 point-GEMMs (contraction C_IN=128 on partitions, W-taps v=0..2
    accumulated in PSUM):  M_p[o, it, j] = sum_v sum_c gw[p,v][c,o] t_p[c,it,j+v]
    with gw = G @ w along u (host-side, fp32 then bf16).
    Outputs:  y[2it]   = m0 + (m1+bias) + m2
              y[2it+1] = (m1+bias) - m2 - m3
  - Engine split per PSUM group (<=8 row-tiles x half of C_OUT):
      PE:  12 matmuls of N=rn*62 cols (4 points x 3 taps)
      ACT: two activations evacuate PSUM->SBUF bf16 — {M1 + per-partition
           bias} (Identity) and {M0,M2,M3} (Copy, one strided AP; PSUM
           point order m1,m0,m2,m3)
      DVE: input transform (4 tensor_tensor) + 4 plain tensor_tensor
           combines (scalar_tensor_tensor has no 2x uop — avoid it)
  - PSUM: 2 groups x [128,4,512] fp32 = 8 banks exactly; groups alternate
    so evacuation overlaps the next group's matmuls.
  - Startup: weights ride Sync (w0 then w1); the first image's first chunk
    rides Scalar as two 10-row slices so the DVE transform overlaps the
    DMA, and chunk0 is processed as 4 half-size groups. Dummy matmuls
    ramp the PE clock until the data lands; a dummy activation preloads
    the ACT table set first.
  - Tail: the last chunk's half1 is split 5+2 so the final
    evac+combine+store chain is short; the last store rides Sync (HWDGE).
  - DMA rings: Sync = w0 + w1 + half0 stores; Scalar = x chunk0 slices +
    piece(0,1) then pure evacuation compute; GpSimd = bias + remaining
    pieces + half1 stores.
"""

import numpy as np
import ml_dtypes

import concourse.bacc as bacc
import concourse.tile as tile
from concourse import mybir
from concourse.bass_utils import run_bass_kernel_spmd

N_CORES = 8
B_FULL, C_IN, H, W = 32, 128, 64, 64
C_OUT, KH, KW = 256, 3, 3
B_LOC = B_FULL // N_CORES          # images per core
H_OUT = W_OUT = H - KH + 1         # 62
N_HALF = C_OUT // 128              # 2 output-channel halves
NT = H_OUT // 2                    # 31 row-tiles (2 output rows each)
CHUNKS = [(0, 8), (8, 8), (16, 8), (24, 7)]   # (first tile, tiles in chunk)
N_PTS = 4                          # F(2,3) points
SLOT = {1: 0, 0: 1, 2: 2, 3: 3}    # PSUM/evac point order: m1, m0, m2, m3
N_WARM = 6                         # DVFS warm-up matmuls
GW_LEN = N_PTS * KW * 128          # 1536 per half

_cached = {}


def _build_nc():
    f32 = mybir.dt.float32
    bf16 = mybir.dt.bfloat16
    AF = mybir.ActivationFunctionType
    ALU = mybir.AluOpType
    nc = bacc.Bacc()

    w0_d = nc.declare_dram_parameter("w0", [C_IN, GW_LEN], bf16, isOutput=False)
    w1_d = nc.declare_dram_parameter("w1", [C_IN, GW_LEN], bf16, isOutput=False)
    x_d = nc.declare_dram_parameter("x", [B_LOC, C_IN, H, W], bf16, isOutput=False)
    b_d = nc.declare_dram_parameter("bias_in", [128, N_HALF], f32, isOutput=False)
    y_d = nc.declare_dram_parameter(
        "y", [B_LOC, N_HALF, 128, H_OUT, W_OUT], bf16, isOutput=True
    )

    with tile.TileContext(nc) as tc:
        with (
            tc.tile_pool(name="const", bufs=1) as cpool,
            tc.tile_pool(name="xin", bufs=5) as xpool,
            tc.tile_pool(name="tin", bufs=8) as tpool,
            tc.tile_pool(name="mev", bufs=4) as mpool,
            tc.tile_pool(name="yout", bufs=5) as ypool,
            tc.tile_pool(name="tmp", bufs=4) as spool,
            tc.tile_pool(name="psum", bufs=2, space="PSUM") as ppool,
        ):
            w0_t = cpool.tile([C_IN, GW_LEN], bf16)
            w1_t = cpool.tile([C_IN, GW_LEN], bf16)
            xa_t = cpool.tile([C_IN, 10, W], bf16)
            xb_t = cpool.tile([C_IN, 10, W], bf16)
            b_t = cpool.tile([128, N_HALF], f32)
            scr = cpool.tile([128, 512], bf16)
            actw = cpool.tile([128, 8], bf16)

            nc.vector.memset(scr[:], 0.0)

            # Startup DMAs. Weights on Sync; img0 rows 0-9 / 8-17 on Scalar
            # (two slices so the first transform overlaps the second DMA);
            # bias leads GpSimd.
            nc.sync.dma_start(w0_t[:], w0_d[:])
            nc.sync.dma_start(w1_t[:], w1_d[:])
            nc.scalar.dma_start(xa_t[:], x_d[0, :, 0:10, :])
            nc.scalar.dma_start(xb_t[:], x_d[0, :, 8:18, :])
            nc.gpsimd.dma_start(b_t[:], b_d[:])

            # Preload the ACT function-table set off the critical path.
            nc.scalar.activation(actw[:], scr[:, 0:8], AF.Copy)

            gw0 = w0_t.rearrange("c (t o) -> c t o", t=N_PTS * KW)
            gw1 = w1_t.rearrange("c (t o) -> c t o", t=N_PTS * KW)

            def lhsT(half, p, v):
                gv = gw0 if half == 0 else gw1
                return gv[:, p * KW + v, :]

            # Ramp the PE p-state while the startup DMAs land. Warm-ups
            # write the first PSUM group buffer (its first real user is the
            # 3rd group, long after these complete). Distinct sizes so no
            # two warm-up matmuls are identical instructions.
            pwarm = ppool.tile([128, N_PTS, 512], f32, tag="ps")
            for i in range(N_WARM):
                nc.tensor.matmul(
                    pwarm[:, i % N_PTS, 0 : 512 - i],
                    scr[:, 0:128],
                    scr[:, 0 : 512 - i],
                    start=True,
                    stop=True,
                )

            piece = {}

            def load_piece(b, c, eng):
                r0 = 16 * c
                r1 = min(r0 + 18, H)
                px = xpool.tile([C_IN, 18, W], bf16, tag="x")
                eng.dma_start(px[:, 0 : r1 - r0, :], x_d[b, :, r0:r1, :])
                piece[(b, c)] = px

            load_piece(0, 1, nc.scalar)
            load_piece(0, 2, nc.gpsimd)
            load_piece(0, 3, nc.gpsimd)

            def transform_rows(px, ctn):
                # DVE input transform: 4 point-tiles for ctn row-tiles.
                ts = [
                    tpool.tile([C_IN, 8, W], bf16, tag="t", name=f"t{p}")
                    for p in range(N_PTS)
                ]
                r = lambda a: px[:, a : a + 2 * ctn - 1 : 2, :]
                nc.vector.tensor_tensor(ts[0][:, 0:ctn, :], r(0), r(2), ALU.subtract)
                nc.vector.tensor_tensor(ts[1][:, 0:ctn, :], r(1), r(2), ALU.add)
                nc.vector.tensor_tensor(ts[2][:, 0:ctn, :], r(2), r(1), ALU.subtract)
                nc.vector.tensor_tensor(ts[3][:, 0:ctn, :], r(1), r(3), ALU.subtract)
                return ts

            def transform(b, c):
                return transform_rows(piece[(b, c)], CHUNKS[c][1])

            def do_group(b, half, it0, rn, ts, tr0, store_eng):
                # One PSUM group: rn row-tiles starting at absolute tile it0,
                # reading ts[p] rows [tr0, tr0+rn).
                n = rn * W_OUT
                ps = ppool.tile([128, N_PTS, 512], f32, tag="ps")
                m = mpool.tile([128, N_PTS, 8 * W_OUT], bf16, tag="m")
                yt = ypool.tile([128, 16, W_OUT], bf16, tag="y")
                bh = b_t[:, half : half + 1]
                for p in range(N_PTS):
                    for v in range(KW):
                        nc.tensor.matmul(
                            ps[:, SLOT[p], 0:n],
                            lhsT(half, p, v),
                            ts[p][:, tr0 : tr0 + rn, v : v + W_OUT],
                            start=(v == 0),
                            stop=(v == KW - 1),
                        )
                # Evacuate PSUM->SBUF bf16: {m1 + bias}, then {m0,m2,m3}.
                nc.scalar.activation(m[:, 0, 0:n], ps[:, 0, 0:n], AF.Identity, bias=bh)
                nc.scalar.activation(m[:, 1:4, 0:n], ps[:, 1:4, 0:n], AF.Copy)
                m3d = [
                    m[:, SLOT[p], 0:n].rearrange("c (h w) -> c h w", w=W_OUT)
                    for p in range(N_PTS)
                ]
                y0 = yt[:, 0 : 2 * rn : 2, :]
                y1 = yt[:, 1 : 2 * rn : 2, :]
                s = spool.tile([128, 8, W_OUT], bf16, tag="s")
                sv = s[:, 0:rn, :]
                nc.vector.tensor_tensor(sv, m3d[0], m3d[1], ALU.add)
                nc.vector.tensor_tensor(y0, sv, m3d[2], ALU.add)
                u = spool.tile([128, 8, W_OUT], bf16, tag="s")
                uv = u[:, 0:rn, :]
                nc.vector.tensor_tensor(uv, m3d[1], m3d[2], ALU.subtract)
                nc.vector.tensor_tensor(y1, uv, m3d[3], ALU.subtract)
                store_eng.dma_start(
                    y_d[b, half, :, 2 * it0 : 2 * (it0 + rn), :],
                    yt[:, 0 : 2 * rn, :],
                )

            # Chunk (0,0) as two 4-tile sub-chunks from the xa/xb slices.
            tsA = transform_rows(xa_t, 4)
            tsB = transform_rows(xb_t, 4)

            order = [(b, c) for b in range(B_LOC) for c in range(len(CHUNKS))]
            ts_cur = None
            for gi, (b, c) in enumerate(order):
                if gi + 2 < len(order) and order[gi + 2] not in piece:
                    load_piece(*order[gi + 2], nc.gpsimd)
                ts_next = transform(*order[gi + 1]) if gi + 1 < len(order) else None
                it0, ctn = CHUNKS[c]
                if gi == 0:
                    do_group(0, 0, 0, 4, tsA, 0, nc.sync)
                    do_group(0, 0, 4, 4, tsB, 0, nc.sync)
                    do_group(0, 1, 0, 4, tsA, 0, nc.gpsimd)
                    do_group(0, 1, 4, 4, tsB, 0, nc.gpsimd)
                elif gi < len(order) - 1:
                    do_group(b, 0, it0, ctn, ts_cur, 0, nc.sync)
                    do_group(b, 1, it0, ctn, ts_cur, 0, nc.gpsimd)
                else:
                    # Tail: h0 whole; h1 split 5+2, last store on Sync.
                    do_group(b, 0, it0, ctn, ts_cur, 0, nc.sync)
                    do_group(b, 1, it0, 5, ts_cur, 0, nc.gpsimd)
                    do_group(b, 1, it0 + 5, 2, ts_cur, 5, nc.sync)
                ts_cur = ts_next

    nc.compile()
    if not nc.is_finalized():
        nc.finalize()
    return nc


_G = np.array(
    [[1.0, 0.0, 0.0], [0.5, 0.5, 0.5], [0.5, -0.5, 0.5], [0.0, 0.0, 1.0]],
    dtype=np.float32,
)


def kernel(inputs, weights, bias, profile=False, trace_kwargs=None):
    x_b = np.ascontiguousarray(
        np.asarray(inputs, dtype=np.float32).astype(ml_dtypes.bfloat16)
    )
    # gw[p,v][c,o]: Winograd-transformed taps, [c, half, p*3+v, o_local]
    w = np.asarray(weights, dtype=np.float32)
    gw = np.einsum("pu,ocuv->cpvo", _G, w)          # [128, 4, 3, 256]
    gwh = (
        gw.reshape(C_IN, N_PTS * KW, N_HALF, 128)
        .transpose(0, 2, 1, 3)
        .astype(ml_dtypes.bfloat16)
    )                                                # [128, 2, 12, 128]
    w0_flat = np.ascontiguousarray(gwh[:, 0].reshape(C_IN, GW_LEN))
    w1_flat = np.ascontiguousarray(gwh[:, 1].reshape(C_IN, GW_LEN))
    # [C_OUT, 1] -> [128, N_HALF] with b_t[p, h] = bias[h*128 + p]
    b_t = np.ascontiguousarray(
        np.asarray(bias, dtype=np.float32).reshape(N_HALF, 128).T
    )

    if "nc" not in _cached:
        _cached["nc"] = _build_nc()
    nc = _cached["nc"]

    in_maps = []
    for i in range(N_CORES):
        shard = x_b[i * B_LOC : (i + 1) * B_LOC]
        in_maps.append(
            {"w0": w0_flat, "w1": w1_flat, "x": shard, "bias_in": b_t}
        )
    res = run_bass_kernel_spmd(
        nc,
        in_maps,
        list(range(N_CORES)),
        trace=profile,
        **(trace_kwargs or {}),
    )
    _cached["last_result"] = res

    shards = []
    for i in range(N_CORES):
        y = res.results[i]["y"]  # [B_LOC, 2, 128, 62, 62] bf16
        shards.append(
            np.asarray(y).astype(np.float32).reshape(B_LOC, C_OUT, H_OUT, W_OUT)
        )
    return np.ascontiguousarray(np.concatenate(shards, axis=0), dtype=np.float32)
